# revision 1
# baseline (speedup 1.0000x reference)
"""Class-balanced segmentation loss on 8 Trainium2 NeuronCores.

Math: with counts_c = #{p: t_p == c}, S_c = sum_{p: t_p=c} logsumexp_p,
T_c = sum_{p: t_p=c} pred[c, p], and w_c = 0.001 / (1 - 0.999**counts_c)
(0 for empty classes), the reference loss is

    loss = sum_c w_c * (S_c - T_c) / sum_c w_c * counts_c .

Everything is linear in w, so the device does a single data-parallel pass
(one batch per core) producing per-core partials (counts, S, T) and the
19-float weight/loss arithmetic runs on the host after the gather.

Device pass per core (pixels on partitions, [128, 2048] per class):
  exp (ACT, one instr per chunk over all 19 classes) ->
  sumexp over classes (DVE tensor_tensor adds) -> log (ACT) ->
  per class fused compare+mul+reduce (scalar_tensor_tensor accum) for
  S_c and T_c, tensor_scalar accum for counts.
"""

import numpy as np

NCLASS = 19
B, H, W = 8, 512, 512
NPIX = H * W          # 262144 pixels per batch
P = 128               # SBUF partitions
FW = NPIX // P        # 2048 free-dim elements per partition
NCORES = 8

USE_BF16 = True
F = 1024 if USE_BF16 else 512   # free-dim chunk size
NCH = FW // F                   # chunks per batch
NSRED_ACT = 7                   # S-reduces on ACT (rest on DVE)

_COMPILED = {}


def _np_io_dtype():
    if USE_BF16:
        import ml_dtypes

        return ml_dtypes.bfloat16
    return np.float32


def _patch_tile_drain():
    """walrus in this container rejects >1 sem-wait on one instruction
    ("Too many sync wait commands"); the tile-exit Drain carries one wait
    per logical processor. Split them into single-wait NOPs."""
    import bass_rust
    import concourse.tile as tile

    if getattr(tile.TileContext, "_drain_patched", False):
        return

    def _drain_and_barrier(self, tick_clock, wait_clock):
        from concourse.tile import ScopedClock

        probe = self.nc.sync.nop(nofuse=True)
        wait_clock.add_sem_waits(
            probe.ins, ScopedClock({None: tick_clock.global_clock})
        )
        si = probe.ins.sync_info
        waits = list(si.on_wait) if si else []
        if si:
            si.on_wait = waits[:1]
        for i in range(1, len(waits)):
            n = self.nc.sync.nop(nofuse=True)
            n.ins.sync_info = bass_rust.SyncInfo(
                on_wait=waits[i : i + 1], on_update=[]
            )
        self.nc.sync.drain()
        self.nc.all_engine_barrier()
        assert self.sems is not None
        popped = self.nc._tile_sem_poison_stack.pop()
        assert popped is self._sem_poison
        self.nc.clear_and_free_semaphores(list(self.sems.allocated().values()))
        self.nc.all_engine_barrier()

    tile.TileContext._drain_and_barrier = _drain_and_barrier
    tile.TileContext._drain_patched = True


def _split_excess_waits(nc, maxw=1):
    """Post-pass: any instruction carrying more than `maxw` sem-waits gets
    the extras moved onto same-engine NOPs inserted right before it (the
    engine executes in order, so semantics are identical)."""
    import bass_rust

    for blk in nc.m.functions[0].blocks:
        insts = list(blk.instructions)
        out = []
        changed = False
        for inst in insts:
            si = inst.sync_info
            if si is not None and si.on_wait and len(si.on_wait) > maxw:
                waits = list(si.on_wait)
                si.on_wait = waits[:maxw]
                extra = waits[maxw:]
                eng = nc.engines[inst.engine]
                for i in range(0, len(extra), maxw):
                    n = eng.nop(nofuse=True)
                    # the nop was appended to the current bb; move it here
                    cur = nc.cur_bb.bb
                    cur_insts = list(cur.instructions)
                    assert cur_insts[-1].name == n.ins.name
                    cur.instructions = cur_insts[:-1]
                    n.ins.sync_info = bass_rust.SyncInfo(
                        on_wait=extra[i : i + maxw], on_update=[]
                    )
                    out.append(n.ins)
                changed = True
            out.append(inst)
        if changed:
            blk.instructions = out


def build_nc(reps: int = 1):
    """Build the per-core Bass program (SPMD: every core runs this on its
    own batch). reps>1 wraps the body in a For_i loop for HW timing.

    Host passes pred pre-transposed to [P, NCH, NCLASS, F] so each chunk
    is one contiguous-per-partition DMA."""
    from contextlib import ExitStack

    import concourse.bass as bass
    import concourse.tile as tile
    from concourse import mybir

    _patch_tile_drain()

    io_dt = mybir.dt.bfloat16 if USE_BF16 else mybir.dt.float32
    cd = mybir.dt.bfloat16 if USE_BF16 else mybir.dt.float32

    nc = bass.Bass()
    pred = nc.declare_dram_parameter(
        "pred", [P, NCH, NCLASS, F], io_dt, isOutput=False
    )
    targ = nc.declare_dram_parameter("targ", [P, FW], io_dt, isOutput=False)
    # per-class results: columns = [S, T, counts]
    out = nc.declare_dram_parameter(
        "out", [NCLASS, 3], mybir.dt.float32, isOutput=True
    )
    # per-partition counts partials (only used when CRED=dve)
    out2 = nc.declare_dram_parameter(
        "out2", [P, NCH * NCLASS], mybir.dt.float32, isOutput=True
    )

    import os
    MMW = int(os.environ.get("MMW", "512"))  # moving cols per matmul
    GPS_SUMEXP = int(os.environ.get("GPS_SUMEXP", "0"))  # adds done on gpsimd
    GPS_MASK = int(os.environ.get("GPS_MASK", "0"))  # masks on gpsimd
    CRED = os.environ.get("CRED", "pe")  # counts reduce: pe | dve

    with tile.TileContext(nc) as tc:
        with ExitStack() as ctx:
            io = ctx.enter_context(tc.tile_pool(name="io", bufs=2))
            work = ctx.enter_context(tc.tile_pool(name="work", bufs=2))
            pp = ctx.enter_context(tc.tile_pool(name="pp", bufs=4))
            acc = ctx.enter_context(tc.tile_pool(name="acc", bufs=1))
            psp = ctx.enter_context(
                tc.tile_pool(name="psp", bufs=1, space="PSUM")
            )

            # place[:, c, :]: [128, NCLASS] with only column c set to 1 —
            # stationary operand that routes a column-sum to PSUM row c
            place = acc.tile([P, NCLASS, NCLASS], cd)
            nc.vector.memset(place[:, :, :], 0.0)
            for c in range(NCLASS):
                nc.vector.memset(place[:, c, c : c + 1], 1.0)

            acc_t = acc.tile([NCLASS, 4], mybir.dt.float32)
            nc.vector.memset(acc_t[:, :], 0.0)
            c_acc = acc.tile([P, NCH * NCLASS], mybir.dt.float32)
            nc.vector.memset(c_acc[:, :], 0.0)

            # per-pass PSUM accumulators [19, F]
            ps = {}
            for q in ("S", "T", "C"):
                ps_tile = psp.tile(
                    [NCLASS, F], mybir.dt.float32, tag=f"ps{q}", name=f"ps{q}"
                )
                ps[q] = ps_tile

            def pe_reduce(q, c, src_ap, k):
                """Accumulate column sums of src_ap [P, F] into ps[q] row c."""
                for s in range(F // MMW):
                    nc.tensor.matmul(
                        ps[q][:, s * MMW : (s + 1) * MMW],
                        place[:, c, :],
                        src_ap[:, s * MMW : (s + 1) * MMW],
                        start=(k == 0 and c == 0),
                        stop=(k == NCH - 1 and c == NCLASS - 1),
                    )

            def _body():
                for k in range(NCH):
                    p_tile = io.tile([P, NCLASS, F], io_dt, tag="p")
                    nc.sync.dma_start(out=p_tile[:, :, :], in_=pred[:, k, :, :])
                    t_tile = io.tile([P, F], io_dt, tag="t")
                    nc.sync.dma_start(
                        out=t_tile[:], in_=targ[:, k * F : (k + 1) * F]
                    )

                    # exp of all classes in one ACT instruction
                    e_tile = work.tile([P, NCLASS, F], cd, tag="e")
                    nc.scalar.activation(
                        out=e_tile[:, :, :],
                        in_=p_tile[:, :, :],
                        func=mybir.ActivationFunctionType.Exp,
                    )
                    # sumexp over classes: chain of small adds, optionally
                    # with a gpsimd side-chain taking GPS_SUMEXP classes
                    sx = work.tile([P, F], cd, tag="sx")
                    ng = GPS_SUMEXP
                    if ng >= 2:
                        sxg = work.tile([P, F], cd, tag="sxg")
                        nc.gpsimd.tensor_tensor(
                            sxg[:], e_tile[:, 0, :], e_tile[:, 1, :],
                            mybir.AluOpType.add,
                        )
                        for c in range(2, ng):
                            nc.gpsimd.tensor_tensor(
                                sxg[:], sxg[:], e_tile[:, c, :],
                                mybir.AluOpType.add,
                            )
                        nc.vector.tensor_tensor(
                            sx[:], e_tile[:, ng, :], e_tile[:, ng + 1, :],
                            mybir.AluOpType.add,
                        )
                        for c in range(ng + 2, NCLASS):
                            nc.vector.tensor_tensor(
                                sx[:], sx[:], e_tile[:, c, :],
                                mybir.AluOpType.add,
                            )
                        nc.vector.tensor_tensor(
                            sx[:], sx[:], sxg[:], mybir.AluOpType.add
                        )
                    else:
                        nc.vector.tensor_tensor(
                            sx[:], e_tile[:, 0, :], e_tile[:, 1, :],
                            mybir.AluOpType.add,
                        )
                        for c in range(2, NCLASS):
                            nc.vector.tensor_tensor(
                                sx[:], sx[:], e_tile[:, c, :],
                                mybir.AluOpType.add,
                            )
                    lse = work.tile([P, F], cd, tag="lse")
                    nc.scalar.activation(
                        out=lse[:],
                        in_=sx[:],
                        func=mybir.ActivationFunctionType.Ln,
                    )

                    for c in range(NCLASS):
                        # mask (plain TS, no accum) — on gpsimd (1-input op
                        # runs near line-rate there; frees the DVE)
                        mask = pp.tile([P, F], cd, tag="mask")
                        mask_eng = nc.gpsimd if GPS_MASK else nc.vector
                        if CRED == "dve":
                            col = k * NCLASS + c
                            nc.vector.tensor_scalar(
                                out=mask[:],
                                in0=t_tile[:],
                                scalar1=float(c),
                                scalar2=0.0,
                                op0=mybir.AluOpType.is_equal,
                                op1=mybir.AluOpType.add,
                                accum_out=c_acc[:, col : col + 1],
                            )
                        else:
                            mask_eng.tensor_scalar(
                                out=mask[:],
                                in0=t_tile[:],
                                scalar1=float(c),
                                scalar2=None,
                                op0=mybir.AluOpType.is_equal,
                            )
                            pe_reduce("C", c, mask[:, :], k)
                        # masked pred / masked lse products
                        prod_t = pp.tile([P, F], cd, tag="prodt")
                        nc.vector.tensor_tensor(
                            prod_t[:], mask[:], p_tile[:, c, :],
                            mybir.AluOpType.mult,
                        )
                        pe_reduce("T", c, prod_t[:, :], k)
                        prod_s = pp.tile([P, F], cd, tag="prods")
                        nc.vector.tensor_tensor(
                            prod_s[:], mask[:], lse[:], mybir.AluOpType.mult
                        )
                        pe_reduce("S", c, prod_s[:, :], k)

                # drain PSUM accumulators: free-reduce [19, F] -> [19, 1]
                qs = ("S", "T") if CRED == "dve" else ("S", "T", "C")
                for i, q in enumerate(qs):
                    junk = work.tile([NCLASS, F], cd, tag=f"junk{q}")
                    nc.scalar.activation(
                        out=junk[:, :],
                        in_=ps[q][:, :],
                        func=mybir.ActivationFunctionType.Copy,
                        accum_out=acc_t[:, i : i + 1],
                    )

            if reps == 1:
                _body()
            else:
                with tc.For_i(0, reps, 1):
                    _body()

            nc.sync.dma_start(out=out[:, :], in_=acc_t[:, 0:3])
            nc.sync.dma_start(out=out2[:, :], in_=c_acc[:, :])

    _split_excess_waits(nc, maxw=1)
    return nc


def _shard_inputs(pred_np, targ_np):
    dt = _np_io_dtype()
    in_maps = []
    for b in range(NCORES):
        # [19, 262144] -> [P, NCH, NCLASS, F]
        pb = pred_np[b].reshape(NCLASS, P, NCH, F).transpose(1, 2, 0, 3)
        in_maps.append(
            {
                "pred": np.ascontiguousarray(pb).astype(dt),
                "targ": targ_np[b].reshape(P, FW).astype(dt),
            }
        )
    return in_maps


def _run_device(pred_np, targ_np, reps: int = 1, in_maps=None):
    """Shard batch-wise over the 8 cores, run the SPMD program, return the
    per-core [P, 3*NCH*19] partial tensors."""
    from concourse.bass_utils import run_bass_kernel_spmd

    if reps not in _COMPILED:
        _COMPILED[reps] = build_nc(reps)
    nc = _COMPILED[reps]

    if in_maps is None:
        in_maps = _shard_inputs(pred_np, targ_np)
    res = run_bass_kernel_spmd(nc, in_maps, core_ids=list(range(NCORES)))
    return [
        (res.results[i]["out"], res.results[i]["out2"]) for i in range(NCORES)
    ]


def _finish(outs):
    """Host epilogue: gather/all-reduce the 3x19 partials and apply the
    class-balanced weight formula (matches reference semantics)."""
    S = np.zeros(NCLASS, np.float64)
    T = np.zeros(NCLASS, np.float64)
    C = np.zeros(NCLASS, np.float64)
    for o, o2 in outs:
        o = np.asarray(o, np.float64)  # [NCLASS, 3]
        o2 = np.asarray(o2, np.float64)  # [P, NCH*NCLASS]
        S += o[:, 0]
        T += o[:, 1]
        C += o[:, 2]
        C += o2.reshape(P, NCH, NCLASS).sum((0, 1))
    beta = 1.0 - 0.001
    with np.errstate(divide="ignore", over="ignore", under="ignore"):
        w = (1.0 - beta) / (1.0 - beta**C)
    w = np.where(C > 0, w, 0.0)
    num = float(np.sum(w * (S - T)))
    den = float(np.sum(w * C))
    return np.array(np.float32(num / den))


def kernel(pred: np.ndarray, target: np.ndarray) -> np.ndarray:
    pred_np = np.asarray(pred, dtype=np.float32)
    targ_np = np.asarray(target)
    outs = _run_device(pred_np, targ_np, reps=1)
    return _finish(outs)



# revision 5
# speedup vs baseline: 5.2036x; 5.2036x over previous
"""Class-balanced segmentation loss on 8 Trainium2 NeuronCores.

Math: with counts_c = #{p: t_p == c} and
NLL_c = sum_{p: t_p=c} (logsumexp_p - pred[c, p]), the reference loss is

    loss = sum_c w_c * NLL_c / sum_c w_c * counts_c,
    w_c = 0.001 / (1 - 0.999**counts_c)   (0 for empty classes).

Everything is linear in w, so the device does a single data-parallel pass
(one batch per core) producing per-core partials (counts, NLL) and the
19-float weight/loss arithmetic runs on the host after the gather.

Device pass per core (pixels on partitions, [128, 2048] per class):
  exp (one ACT instr per chunk over all 19 classes) ->
  sumexp over classes (DVE pairwise tree, 6 large TT adds) -> log (ACT) ->
  nll[c] = lse - pred[c] for all classes in ONE broadcast TT ->
  per class one fused STT (t==c)*nll[c] plus one TS mask, both reduced
  on the PE via a one-hot 'place' stationary into PSUM accumulators.
"""

import numpy as np

NCLASS = 19
B, H, W = 8, 512, 512
NPIX = H * W          # 262144 pixels per batch
P = 128               # SBUF partitions
FW = NPIX // P        # 2048 free-dim elements per partition
NCORES = 8

USE_BF16 = True
F = 1024              # free-dim chunk size
NCH = FW // F         # chunks per batch

_COMPILED = {}


def _np_io_dtype():
    if USE_BF16:
        import ml_dtypes

        return ml_dtypes.bfloat16
    return np.float32


def _patch_tile_drain():
    """walrus in this container rejects >1 sem-wait on one instruction
    ("Too many sync wait commands"); the tile-exit Drain carries one wait
    per logical processor. Split them into single-wait NOPs."""
    import bass_rust
    import concourse.tile as tile

    if getattr(tile.TileContext, "_drain_patched", False):
        return

    def _drain_and_barrier(self, tick_clock, wait_clock):
        from concourse.tile import ScopedClock

        probe = self.nc.sync.nop(nofuse=True)
        wait_clock.add_sem_waits(
            probe.ins, ScopedClock({None: tick_clock.global_clock})
        )
        si = probe.ins.sync_info
        waits = list(si.on_wait) if si else []
        if si:
            si.on_wait = waits[:1]
        for i in range(1, len(waits)):
            n = self.nc.sync.nop(nofuse=True)
            n.ins.sync_info = bass_rust.SyncInfo(
                on_wait=waits[i : i + 1], on_update=[]
            )
        self.nc.sync.drain()
        self.nc.all_engine_barrier()
        assert self.sems is not None
        popped = self.nc._tile_sem_poison_stack.pop()
        assert popped is self._sem_poison
        self.nc.clear_and_free_semaphores(list(self.sems.allocated().values()))
        self.nc.all_engine_barrier()

    tile.TileContext._drain_and_barrier = _drain_and_barrier
    tile.TileContext._drain_patched = True


def _split_excess_waits(nc, maxw=1):
    """Post-pass: any instruction carrying more than `maxw` sem-waits gets
    the extras moved onto same-engine NOPs inserted right before it (the
    engine executes in order, so semantics are identical)."""
    import bass_rust

    for blk in nc.m.functions[0].blocks:
        insts = list(blk.instructions)
        out = []
        changed = False
        for inst in insts:
            si = inst.sync_info
            if si is not None and si.on_wait and len(si.on_wait) > maxw:
                waits = list(si.on_wait)
                si.on_wait = waits[:maxw]
                extra = waits[maxw:]
                eng = nc.engines[inst.engine]
                for i in range(0, len(extra), maxw):
                    n = eng.nop(nofuse=True)
                    # the nop was appended to the current bb; move it here
                    cur = nc.cur_bb.bb
                    cur_insts = list(cur.instructions)
                    assert cur_insts[-1].name == n.ins.name
                    cur.instructions = cur_insts[:-1]
                    n.ins.sync_info = bass_rust.SyncInfo(
                        on_wait=extra[i : i + maxw], on_update=[]
                    )
                    out.append(n.ins)
                changed = True
            out.append(inst)
        if changed:
            blk.instructions = out


def build_nc(reps: int = 1):
    """Build the per-core Bass program (SPMD: every core runs this on its
    own batch). reps>1 wraps the body in a For_i loop for HW timing.

    Host passes pred pre-transposed to [P, NCH, NCLASS, F] so each chunk
    is one contiguous-per-partition DMA."""
    import os
    from contextlib import ExitStack

    import concourse.bass as bass
    import concourse.tile as tile
    from concourse import mybir

    _patch_tile_drain()

    io_dt = mybir.dt.bfloat16 if USE_BF16 else mybir.dt.float32
    cd = mybir.dt.bfloat16 if USE_BF16 else mybir.dt.float32

    nc = bass.Bass()
    pred = nc.declare_dram_parameter(
        "pred", [P, NCH, NCLASS, F], io_dt, isOutput=False
    )
    targ = nc.declare_dram_parameter("targ", [P, FW], io_dt, isOutput=False)
    # per-class results: columns = [NLL, counts]
    out = nc.declare_dram_parameter(
        "out", [NCLASS, 2], mybir.dt.float32, isOutput=True
    )

    MMW = int(os.environ.get("MMW", "512"))  # moving cols per matmul

    with tile.TileContext(nc) as tc:
        with ExitStack() as ctx:
            io = ctx.enter_context(tc.tile_pool(name="io", bufs=2))
            work = ctx.enter_context(tc.tile_pool(name="work", bufs=2))
            sxp = ctx.enter_context(tc.tile_pool(name="sxp", bufs=1))
            pp = ctx.enter_context(tc.tile_pool(name="pp", bufs=4))
            acc = ctx.enter_context(tc.tile_pool(name="acc", bufs=1))
            psp = ctx.enter_context(
                tc.tile_pool(name="psp", bufs=1, space="PSUM")
            )

            # place[:, c, :]: [128, NCLASS] with only column c set to 1 —
            # stationary operand that routes a column-sum to PSUM row c
            place = acc.tile([P, NCLASS, NCLASS], cd)
            nc.vector.memset(place[:, :, :], 0.0)
            for c in range(NCLASS):
                nc.vector.memset(place[:, c, c : c + 1], 1.0)

            acc_t = acc.tile([NCLASS, 2], mybir.dt.float32)
            nc.vector.memset(acc_t[:, :], 0.0)

            # per-pass PSUM accumulators [19, F]
            ps = {}
            for q in ("S", "C"):
                ps[q] = psp.tile(
                    [NCLASS, F], mybir.dt.float32, tag=f"ps{q}", name=f"ps{q}"
                )

            def pe_reduce(q, c, src_ap, k):
                """Accumulate column sums of src_ap [P, F] into ps[q] row c."""
                for s in range(F // MMW):
                    nc.tensor.matmul(
                        ps[q][:, s * MMW : (s + 1) * MMW],
                        place[:, c, :],
                        src_ap[:, s * MMW : (s + 1) * MMW],
                        start=(k == 0 and c == 0),
                        stop=(k == NCH - 1 and c == NCLASS - 1),
                    )

            def _body():
                for k in range(NCH):
                    p_tile = io.tile([P, NCLASS, F], io_dt, tag="p")
                    nc.sync.dma_start(out=p_tile[:, :, :], in_=pred[:, k, :, :])
                    t_tile = io.tile([P, F], io_dt, tag="t")
                    nc.sync.dma_start(
                        out=t_tile[:], in_=targ[:, k * F : (k + 1) * F]
                    )

                    # exp of all classes in one ACT instruction
                    e_tile = work.tile([P, NCLASS, F], cd, tag="e")
                    nc.scalar.activation(
                        out=e_tile[:, :, :],
                        in_=p_tile[:, :, :],
                        func=mybir.ActivationFunctionType.Exp,
                    )
                    # sumexp over classes: pairwise tree, 6 large TT adds
                    # (bf16 step-1 keeps the 2x DVE mode). Upper levels
                    # write into e_tile slots whose exp values are dead.
                    sx = sxp.tile([P, 9, F], cd, tag="sx")
                    nc.vector.tensor_tensor(
                        sx[:, 0:9, :], e_tile[:, 0:9, :], e_tile[:, 9:18, :],
                        mybir.AluOpType.add,
                    )
                    nc.vector.tensor_tensor(
                        e_tile[:, 0:4, :], sx[:, 0:4, :], sx[:, 4:8, :],
                        mybir.AluOpType.add,
                    )
                    nc.vector.tensor_tensor(
                        e_tile[:, 4:6, :], e_tile[:, 0:2, :], e_tile[:, 2:4, :],
                        mybir.AluOpType.add,
                    )
                    nc.vector.tensor_tensor(
                        e_tile[:, 6, :], e_tile[:, 4, :], e_tile[:, 5, :],
                        mybir.AluOpType.add,
                    )
                    nc.vector.tensor_tensor(
                        e_tile[:, 7, :], e_tile[:, 6, :], sx[:, 8, :],
                        mybir.AluOpType.add,
                    )
                    nc.vector.tensor_tensor(
                        e_tile[:, 8, :], e_tile[:, 7, :], e_tile[:, 18, :],
                        mybir.AluOpType.add,
                    )
                    lse = work.tile([P, F], cd, tag="lse")
                    nc.scalar.activation(
                        out=lse[:],
                        in_=e_tile[:, 8, :],
                        func=mybir.ActivationFunctionType.Ln,
                    )

                    # nll[c] = lse - pred[c] for all classes in ONE TT
                    # (lse broadcast over the class axis via stride-0 AP;
                    # innermost step stays 1 so the 2x mode holds).
                    # Overwrites e_tile - exp values are dead after the tree.
                    nll = e_tile
                    lse_b = lse[:].unsqueeze(1).broadcast_to([P, NCLASS, F])
                    nc.vector.tensor_tensor(
                        nll[:, :, :], lse_b, p_tile[:, :, :],
                        mybir.AluOpType.subtract,
                    )

                    for c in range(NCLASS):
                        # fused mask+product: (t == c) * nll[c] in one STT
                        prod_n = pp.tile([P, F], cd, tag="prodn")
                        nc.vector.scalar_tensor_tensor(
                            out=prod_n[:],
                            in0=t_tile[:],
                            scalar=float(c),
                            in1=nll[:, c, :],
                            op0=mybir.AluOpType.is_equal,
                            op1=mybir.AluOpType.mult,
                        )
                        pe_reduce("S", c, prod_n[:, :], k)
                        # mask for counts (TS, 4x mode)
                        mask = pp.tile([P, F], cd, tag="mask")
                        nc.vector.tensor_scalar(
                            out=mask[:],
                            in0=t_tile[:],
                            scalar1=float(c),
                            scalar2=None,
                            op0=mybir.AluOpType.is_equal,
                        )
                        pe_reduce("C", c, mask[:, :], k)

                # drain PSUM accumulators: free-reduce [19, F] -> [19, 1]
                for i, q in enumerate(("S", "C")):
                    junk = work.tile([NCLASS, F], cd, tag=f"junk{q}")
                    nc.scalar.activation(
                        out=junk[:, :],
                        in_=ps[q][:, :],
                        func=mybir.ActivationFunctionType.Copy,
                        accum_out=acc_t[:, i : i + 1],
                    )

            if reps == 1:
                _body()
            else:
                with tc.For_i(0, reps, 1):
                    _body()

            nc.sync.dma_start(out=out[:, :], in_=acc_t[:, 0:2])

    _split_excess_waits(nc, maxw=1)
    return nc


def _shard_inputs(pred_np, targ_np):
    dt = _np_io_dtype()
    in_maps = []
    for b in range(NCORES):
        # [19, 262144] -> [P, NCH, NCLASS, F]
        pb = pred_np[b].reshape(NCLASS, P, NCH, F).transpose(1, 2, 0, 3)
        in_maps.append(
            {
                "pred": np.ascontiguousarray(pb).astype(dt),
                "targ": targ_np[b].reshape(P, FW).astype(dt),
            }
        )
    return in_maps


def _run_device(pred_np, targ_np, reps: int = 1, in_maps=None):
    """Shard batch-wise over the 8 cores, run the SPMD program, return the
    per-core [NCLASS, 2] partial tensors."""
    from concourse.bass_utils import run_bass_kernel_spmd

    if reps not in _COMPILED:
        _COMPILED[reps] = build_nc(reps)
    nc = _COMPILED[reps]

    if in_maps is None:
        in_maps = _shard_inputs(pred_np, targ_np)
    res = run_bass_kernel_spmd(nc, in_maps, core_ids=list(range(NCORES)))
    return [res.results[i]["out"] for i in range(NCORES)]


def _finish(outs):
    """Host epilogue: gather/all-reduce the 2x19 partials and apply the
    class-balanced weight formula (matches reference semantics)."""
    N = np.zeros(NCLASS, np.float64)
    C = np.zeros(NCLASS, np.float64)
    for o in outs:
        o = np.asarray(o, np.float64)  # [NCLASS, 2]
        N += o[:, 0]
        C += o[:, 1]
    beta = 1.0 - 0.001
    with np.errstate(divide="ignore", over="ignore", under="ignore"):
        w = (1.0 - beta) / (1.0 - beta**C)
    w = np.where(C > 0, w, 0.0)
    num = float(np.sum(w * N))
    den = float(np.sum(w * C))
    return np.array(np.float32(num / den))


def kernel(pred: np.ndarray, target: np.ndarray) -> np.ndarray:
    pred_np = np.asarray(pred, dtype=np.float32)
    targ_np = np.asarray(target)
    outs = _run_device(pred_np, targ_np, reps=1)
    return _finish(outs)


# revision 12
# speedup vs baseline: 8.3351x; 1.6018x over previous
"""Class-balanced segmentation loss on 8 Trainium2 NeuronCores.

Math: with counts_c = #{p: t_p == c} and
NLL_c = sum_{p: t_p=c} (logsumexp_p - pred[c, p]), the reference loss is

    loss = sum_c w_c * NLL_c / sum_c w_c * counts_c,
    w_c = 0.001 / (1 - 0.999**counts_c)   (0 for empty classes).

Everything is linear in w, so the device does a single data-parallel pass
(one batch per core) producing per-core partials (counts, NLL) and the
19-float weight/loss arithmetic runs on the host after the gather.

Device pass per core (pixels on partitions, [128, 2048] per class):
  exp (one ACT instr per chunk over all 19 classes) ->
  sumexp over classes (DVE pairwise tree, 6 large TT adds) -> log (ACT) ->
  nll[c] = lse - pred[c] for all classes in ONE broadcast TT ->
  per class one fused STT (t==c)*nll[c] plus one TS mask, both reduced
  on the PE via a one-hot 'place' stationary into PSUM accumulators.
"""

import numpy as np

NCLASS = 19
B, H, W = 8, 512, 512
NPIX = H * W          # 262144 pixels per batch
P = 128               # SBUF partitions
FW = NPIX // P        # 2048 free-dim elements per partition
NCORES = 8

USE_BF16 = True
F = 512               # free-dim chunk size
NCH = FW // F         # chunks per batch

_COMPILED = {}


def _np_io_dtype():
    if USE_BF16:
        import ml_dtypes

        return ml_dtypes.bfloat16
    return np.float32


def _patch_tile_drain():
    """walrus in this container rejects >1 sem-wait on one instruction
    ("Too many sync wait commands"); the tile-exit Drain carries one wait
    per logical processor. Split them into single-wait NOPs."""
    import bass_rust
    import concourse.tile as tile

    if getattr(tile.TileContext, "_drain_patched", False):
        return

    def _drain_and_barrier(self, tick_clock, wait_clock):
        from concourse.tile import ScopedClock

        probe = self.nc.sync.nop(nofuse=True)
        wait_clock.add_sem_waits(
            probe.ins, ScopedClock({None: tick_clock.global_clock})
        )
        si = probe.ins.sync_info
        waits = list(si.on_wait) if si else []
        if si:
            si.on_wait = waits[:1]
        for i in range(1, len(waits)):
            n = self.nc.sync.nop(nofuse=True)
            n.ins.sync_info = bass_rust.SyncInfo(
                on_wait=waits[i : i + 1], on_update=[]
            )
        self.nc.sync.drain()
        self.nc.all_engine_barrier()
        assert self.sems is not None
        popped = self.nc._tile_sem_poison_stack.pop()
        assert popped is self._sem_poison
        self.nc.clear_and_free_semaphores(list(self.sems.allocated().values()))
        self.nc.all_engine_barrier()

    tile.TileContext._drain_and_barrier = _drain_and_barrier
    tile.TileContext._drain_patched = True


def _split_excess_waits(nc, maxw=1):
    """Post-pass: any instruction carrying more than `maxw` sem-waits gets
    the extras moved onto same-engine NOPs inserted right before it (the
    engine executes in order, so semantics are identical)."""
    import bass_rust

    for blk in nc.m.functions[0].blocks:
        insts = list(blk.instructions)
        out = []
        changed = False
        for inst in insts:
            si = inst.sync_info
            if si is not None and si.on_wait and len(si.on_wait) > maxw:
                waits = list(si.on_wait)
                si.on_wait = waits[:maxw]
                extra = waits[maxw:]
                eng = nc.engines[inst.engine]
                for i in range(0, len(extra), maxw):
                    n = eng.nop(nofuse=True)
                    # the nop was appended to the current bb; move it here
                    cur = nc.cur_bb.bb
                    cur_insts = list(cur.instructions)
                    assert cur_insts[-1].name == n.ins.name
                    cur.instructions = cur_insts[:-1]
                    n.ins.sync_info = bass_rust.SyncInfo(
                        on_wait=extra[i : i + maxw], on_update=[]
                    )
                    out.append(n.ins)
                changed = True
            out.append(inst)
        if changed:
            blk.instructions = out


def build_nc(reps: int = 1):
    """Build the per-core Bass program (SPMD: every core runs this on its
    own batch). reps>1 wraps the body in a For_i loop for HW timing.

    Host passes pred pre-transposed to [P, NCH, NCLASS, F] so each chunk
    is one contiguous-per-partition DMA."""
    import os
    from contextlib import ExitStack

    import concourse.bass as bass
    import concourse.tile as tile
    from concourse import mybir

    _patch_tile_drain()

    io_dt = mybir.dt.bfloat16 if USE_BF16 else mybir.dt.float32
    cd = mybir.dt.bfloat16 if USE_BF16 else mybir.dt.float32

    nc = bass.Bass()
    pred = nc.declare_dram_parameter(
        "pred", [P, NCH, NCLASS, F], io_dt, isOutput=False
    )
    targ = nc.declare_dram_parameter("targ", [P, FW], io_dt, isOutput=False)
    # per-class results: columns = [NLL, counts]
    out = nc.declare_dram_parameter(
        "out", [NCLASS, 2], mybir.dt.float32, isOutput=True
    )

    MMW = int(os.environ.get("MMW", "512"))  # moving cols per matmul

    with tile.TileContext(nc) as tc:
        with ExitStack() as ctx:
            io = ctx.enter_context(tc.tile_pool(name="io", bufs=2))
            work = ctx.enter_context(tc.tile_pool(name="work", bufs=2))
            sxp = ctx.enter_context(tc.tile_pool(name="sxp", bufs=1))
            mkp = ctx.enter_context(tc.tile_pool(name="mkp", bufs=2))
            pp = ctx.enter_context(tc.tile_pool(name="pp", bufs=2))
            acc = ctx.enter_context(tc.tile_pool(name="acc", bufs=1))
            psp = ctx.enter_context(
                tc.tile_pool(name="psp", bufs=1, space="PSUM")
            )

            # place[:, c, :]: [128, NCLASS] with only column c set to 1 —
            # stationary operand that routes a column-sum to PSUM row c
            place = acc.tile([P, NCLASS, NCLASS], cd)
            nc.vector.memset(place[:, :, :], 0.0)
            for c in range(NCLASS):
                nc.vector.memset(place[:, c, c : c + 1], 1.0)

            acc_t = acc.tile([NCLASS, 2], mybir.dt.float32)
            nc.vector.memset(acc_t[:, :], 0.0)

            # per-pass PSUM accumulators [19, F]
            ps = {}
            for q in ("S", "C"):
                ps[q] = psp.tile(
                    [NCLASS, F], mybir.dt.float32, tag=f"ps{q}", name=f"ps{q}"
                )

            def pe_reduce(q, c, src_ap, k):
                """Accumulate column sums of src_ap [P, F] into ps[q] row c."""
                for s in range(F // MMW):
                    nc.tensor.matmul(
                        ps[q][:, s * MMW : (s + 1) * MMW],
                        place[:, c, :],
                        src_ap[:, s * MMW : (s + 1) * MMW],
                        start=(k == 0 and c == 0),
                        stop=(k == NCH - 1 and c == NCLASS - 1),
                    )

            def _body():
                # target for the whole batch in one small DMA (4KB/part)
                t_full = io.tile([P, FW], io_dt, tag="t")
                nc.sync.dma_start(out=t_full[:, :], in_=targ[:, :])

                def make_masks(k):
                    """All 19 class masks for chunk k (TS is_equal, 4x) -
                    depends only on t_full, so this runs while DMA/exp of
                    the same chunk are still in flight."""
                    m = mkp.tile([P, NCLASS, F], cd, tag="m")
                    for c in range(NCLASS):
                        nc.vector.tensor_scalar(
                            out=m[:, c, :],
                            in0=t_full[:, k * F : (k + 1) * F],
                            scalar1=float(c),
                            scalar2=None,
                            op0=mybir.AluOpType.is_equal,
                        )
                        pe_reduce("C", c, m[:, c, :], k)
                    return m

                masks = make_masks(0)
                for k in range(NCH):
                    p_tile = io.tile([P, NCLASS, F], io_dt, tag="p")
                    nc.sync.dma_start(out=p_tile[:, :, :], in_=pred[:, k, :, :])

                    # exp of all classes in one ACT instruction
                    e_tile = work.tile([P, NCLASS, F], cd, tag="e")
                    nc.scalar.activation(
                        out=e_tile[:, :, :],
                        in_=p_tile[:, :, :],
                        func=mybir.ActivationFunctionType.Exp,
                    )
                    # sumexp over classes: pairwise tree, 6 large TT adds
                    # (bf16 step-1 keeps the 2x DVE mode). Upper levels
                    # write into e_tile slots whose exp values are dead.
                    sx = sxp.tile([P, 9, F], cd, tag="sx")
                    nc.vector.tensor_tensor(
                        sx[:, 0:9, :], e_tile[:, 0:9, :], e_tile[:, 9:18, :],
                        mybir.AluOpType.add,
                    )
                    nc.vector.tensor_tensor(
                        e_tile[:, 0:4, :], sx[:, 0:4, :], sx[:, 4:8, :],
                        mybir.AluOpType.add,
                    )
                    nc.vector.tensor_tensor(
                        e_tile[:, 4:6, :], e_tile[:, 0:2, :], e_tile[:, 2:4, :],
                        mybir.AluOpType.add,
                    )
                    nc.vector.tensor_tensor(
                        e_tile[:, 6, :], e_tile[:, 4, :], e_tile[:, 5, :],
                        mybir.AluOpType.add,
                    )
                    nc.vector.tensor_tensor(
                        e_tile[:, 7, :], e_tile[:, 6, :], sx[:, 8, :],
                        mybir.AluOpType.add,
                    )
                    nc.vector.tensor_tensor(
                        e_tile[:, 8, :], e_tile[:, 7, :], e_tile[:, 18, :],
                        mybir.AluOpType.add,
                    )
                    lse = work.tile([P, F], cd, tag="lse")
                    nc.scalar.activation(
                        out=lse[:],
                        in_=e_tile[:, 8, :],
                        func=mybir.ActivationFunctionType.Ln,
                    )

                    # masks for the NEXT chunk: fills DVE while ACT works
                    next_masks = make_masks(k + 1) if k + 1 < NCH else None

                    # nll[c] = lse - pred[c] for all classes in ONE TT
                    # (lse broadcast over the class axis via stride-0 AP;
                    # innermost step stays 1 so the 2x mode holds).
                    # Overwrites e_tile - exp values are dead after the tree.
                    nll = e_tile
                    lse_b = lse[:].unsqueeze(1).broadcast_to([P, NCLASS, F])
                    nc.vector.tensor_tensor(
                        nll[:, :, :], lse_b, p_tile[:, :, :],
                        mybir.AluOpType.subtract,
                    )

                    # all 19 products in ONE TT (masks and nll are both
                    # [P, 19, F] contiguous tiles; 2x mode, one instr)
                    prod_n = pp.tile([P, NCLASS, F], cd, tag="prodn")
                    nc.vector.tensor_tensor(
                        prod_n[:, :, :], masks[:, :, :], nll[:, :, :],
                        mybir.AluOpType.mult,
                    )
                    for c in range(NCLASS):
                        pe_reduce("S", c, prod_n[:, c, :], k)
                    masks = next_masks

                # drain PSUM accumulators: free-reduce [19, F] -> [19, 1]
                for i, q in enumerate(("S", "C")):
                    junk = work.tile([NCLASS, F], cd, tag=f"junk{q}")
                    nc.scalar.activation(
                        out=junk[:, :],
                        in_=ps[q][:, :],
                        func=mybir.ActivationFunctionType.Copy,
                        accum_out=acc_t[:, i : i + 1],
                    )

            if reps == 1:
                _body()
            elif os.environ.get("SIM_UNROLL", "0") == "1":
                for _ in range(reps):
                    _body()
            else:
                with tc.For_i(0, reps, 1):
                    _body()

            nc.sync.dma_start(out=out[:, :], in_=acc_t[:, 0:2])

    _split_excess_waits(nc, maxw=1)
    return nc


def _shard_inputs(pred_np, targ_np):
    dt = _np_io_dtype()
    in_maps = []
    for b in range(NCORES):
        # [19, 262144] -> [P, NCH, NCLASS, F]
        pb = pred_np[b].reshape(NCLASS, P, NCH, F).transpose(1, 2, 0, 3)
        in_maps.append(
            {
                "pred": np.ascontiguousarray(pb).astype(dt),
                "targ": targ_np[b].reshape(P, FW).astype(dt),
            }
        )
    return in_maps


def _run_device(pred_np, targ_np, reps: int = 1, in_maps=None):
    """Shard batch-wise over the 8 cores, run the SPMD program, return the
    per-core [NCLASS, 2] partial tensors."""
    from concourse.bass_utils import run_bass_kernel_spmd

    if reps not in _COMPILED:
        _COMPILED[reps] = build_nc(reps)
    nc = _COMPILED[reps]

    if in_maps is None:
        in_maps = _shard_inputs(pred_np, targ_np)
    res = run_bass_kernel_spmd(nc, in_maps, core_ids=list(range(NCORES)))
    return [res.results[i]["out"] for i in range(NCORES)]


def _finish(outs):
    """Host epilogue: gather/all-reduce the 2x19 partials and apply the
    class-balanced weight formula (matches reference semantics)."""
    N = np.zeros(NCLASS, np.float64)
    C = np.zeros(NCLASS, np.float64)
    for o in outs:
        o = np.asarray(o, np.float64)  # [NCLASS, 2]
        N += o[:, 0]
        C += o[:, 1]
    beta = 1.0 - 0.001
    with np.errstate(divide="ignore", over="ignore", under="ignore"):
        w = (1.0 - beta) / (1.0 - beta**C)
    w = np.where(C > 0, w, 0.0)
    num = float(np.sum(w * N))
    den = float(np.sum(w * C))
    return np.array(np.float32(num / den))


def kernel(pred: np.ndarray, target: np.ndarray) -> np.ndarray:
    pred_np = np.asarray(pred, dtype=np.float32)
    targ_np = np.asarray(target)
    outs = _run_device(pred_np, targ_np, reps=1)
    return _finish(outs)
